# revision 11
# baseline (speedup 1.0000x reference)
"""Trainium2 Bass kernel for nn_CrossAttention_9174050144362.

Reference computation (per batch b, spatial flattened to hw=4096):
    Q = Wq @ a + bq      [128, 4096]
    K = Wk @ p + bk      [128, 4096]
    V = Wv @ p + bv      [256, 4096]
    attn = softmax_n(Q^T K)            [4096, 4096]
    out  = V @ attn^T + a              [256, 4096]

Sharding: 8 cores = (4 batches) x (2 query halves of 2048). Each core
computes full K/V for its batch and attends its 2048 queries against all
4096 keys. No collectives.

v6 schedule (from v5):
  * All matmul operands bf16 (host-cast): 3.2MB input stream, 113ns
    LDWEIGHTS, rel err ~3e-3 (vs 2e-2 budget).
  * Chunk 0's VP is folded into the prefix: vp(0, 4(e-2)..4(e-2)+3)
    rides each eighth-iteration, so the input-DMA-bound prefix idle
    fills with phase-0 work; phase 0 keeps only tiles 24..31.
  * Rings: sync [wqk, p q0, q2], gpsimd [p q1, q3] (pure p - quarter 1
    no longer lands behind wvt), scalar [bias, wvt, a x3].
  * S tiles exp'd in TRIPLES ([P,1536], 11 groups/chunk): ACT ~65us.
  * Final phase runs co-major (all co0 VP matmuls, then co1) so the
    co0 epilogue + out DMA hide under co1's matmuls.
  * Denominator: 6 wide out-of-place bf16 adds into a scratch, den
    matmul + 1/x mid-phase; boundary = ep_mul only, covered by the
    next chunk's deferred S groups. PSUM: 2 S slots x 3 + out x 2 = 8.
"""

import numpy as np
import ml_dtypes

import concourse.bass as bass
import concourse.tile as tile
from concourse import bacc, mybir
from concourse.bass_utils import run_bass_kernel_spmd

B, C, H, W = 4, 256, 64, 64
HW = H * W            # 4096 keys
CH = C // 2           # 128 q/k channels
P = 128               # partitions
MS = HW // 2          # 2048 queries per core
MCH = 512             # query chunk (PSUM-bank sized)
NT = HW // P          # 32 key tiles
NCH = MS // MCH       # 4 query chunks
NG = (NT + 2) // 3    # 11 S-exp groups per chunk (3 tiles each, last 2)
NCORES = 8

F32 = mybir.dt.float32
BF16 = mybir.dt.bfloat16
AF = mybir.ActivationFunctionType
BT = ml_dtypes.bfloat16

TRACE = False
TMPDIR = None
LAST_RESULT = None

_PROG = None


def _emit(tc, out_d, a_d, p_d, wqk_d, wvt_d, bias_d):
    nc = tc.nc
    ts = bass.ts

    with (
        tc.tile_pool(name="statics", bufs=1) as statics,
        tc.tile_pool(name="rcp", bufs=2) as rcp,
        tc.tile_pool(name="osb", bufs=4) as osb,
        tc.tile_pool(name="psS", bufs=2, space="PSUM") as psS,
        tc.tile_pool(name="psOut", bufs=1, space="PSUM") as psOut,
    ):
        # ---- statics: wqk at the sync head, wvt + biases on scalar
        wqk_sb = statics.tile([P, 2, 2 * CH], BF16)
        nc.sync.dma_start(wqk_sb[:], wqk_d[:])
        bias_sb = statics.tile([P, 4], F32)
        nc.scalar.dma_start(bias_sb[:], bias_d[:])
        wvt_sb = statics.tile([P, 2, C], BF16)
        nc.scalar.dma_start(wvt_sb[:], wvt_d[:])
        ones_sb = statics.tile([P, P], BF16)
        nc.gpsimd.memset(ones_sb[:], 1.0)

        # ---- bulk inputs (bf16). p quarters alternate sync/gpsimd
        # rings; a in 3 pieces (512, 512, 1024 queries) on scalar.
        a_sb = statics.tile([P, 2, MS], BF16)
        p_sb = statics.tile([P, 2, HW], BF16)
        Q4 = HW // 4
        for i in range(4):
            q = nc.sync if i % 2 == 0 else nc.gpsimd
            q.dma_start(p_sb[:, :, ts(i, Q4)], p_d[:, :, ts(i, Q4)])
        nc.scalar.dma_start(a_sb[:, :, 0:MCH], a_d[:, :, 0:MCH])
        nc.scalar.dma_start(a_sb[:, :, MCH:2 * MCH], a_d[:, :, MCH:2 * MCH])
        nc.scalar.dma_start(a_sb[:, :, 2 * MCH:MS], a_d[:, :, 2 * MCH:MS])

        # ---- persistent SBUF state
        q_sb = statics.tile([P, MS], BF16)
        k_sb = statics.tile([P, HW], BF16)
        vt_sb = statics.tile([P, NT, C], BF16)
        pt_sb = statics.tile([P, 2, NT, MCH], BF16)
        dscr = statics.tile([P, 8, MCH], BF16)     # den tree scratch
        ab2_sb = statics.tile([P, 2, MS], BF16)    # a + bv (residual)

        out_v = out_d.rearrange("(co ci) m -> ci co m", ci=P)

        # ---------- helpers ----------
        def proj_q(c, half):
            for co in range(2):
                nc.tensor.matmul(half[:], wqk_sb[:, co, 0:CH],
                                 a_sb[:, co, ts(c, MCH)],
                                 start=(co == 0), stop=(co == 1))
            nc.vector.tensor_scalar_add(q_sb[:, ts(c, MCH)], half[:],
                                        bias_sb[:, 0:1])

        def proj_k(e, half):
            E8 = HW // 8
            for co in range(2):
                nc.tensor.matmul(half[:], wqk_sb[:, co, CH:2 * CH],
                                 p_sb[:, co, ts(e, E8)],
                                 start=(co == 0), stop=(co == 1))
            nc.vector.tensor_scalar_add(k_sb[:, ts(e, E8)], half[:],
                                        bias_sb[:, 1:2])

        def proj_vt_quad(e, vq):
            # V^T tiles 4e..4e+3 into banks 1-2 of a psS slot; ONE DVE
            # eviction. vt[n, c] = sum_ci p[ci, n] * WvT[ci, c]  (no bias;
            # bv folded into ab2 since attn rows sum to 1)
            vq_f = vq.rearrange("p a b -> p (a b)")
            for i in range(4):
                t = 4 * e + i
                for co in range(2):
                    nc.tensor.matmul(vq_f[:, ts(i, C)], p_sb[:, co, ts(t, P)],
                                     wvt_sb[:, co, :],
                                     start=(co == 0), stop=(co == 1))
            nc.vector.tensor_copy(
                vt_sb[:, 4 * e : 4 * e + 4, :],
                vq_f.rearrange("p (t c) -> p t c", t=4))

        def s_group(c, g):
            # S tiles 3g..3g+2 -> one [P,3,512] PSUM slot, ONE exp over
            # [P,1536] into the pt store.
            s = c % 2
            t0, t1 = 3 * g, min(3 * g + 3, NT)
            ps = psS.tile([P, 3, MCH], F32, tag="ps", name="ps_s")
            for t in range(t0, t1):
                nc.tensor.matmul(ps[:, t - t0, :], k_sb[:, ts(t, P)],
                                 q_sb[:, ts(c, MCH)], start=True, stop=True)
            nc.scalar.activation(pt_sb[:, s, t0:t1, :], ps[:, 0:t1 - t0, :],
                                 AF.Exp)

        def vp(c, t, out_ps, co=None, last=False):
            s = c % 2
            for co_ in ([0, 1] if co is None else [co]):
                nc.tensor.matmul(out_ps[:, co_, :],
                                 vt_sb[:, t, ts(co_, P)], pt_sb[:, s, t, :],
                                 start=(t == 0), stop=last)

        def den_tree(c):
            # dscr[0:8] = sum of the 4 tile-octs (pure reads of pt),
            # then fold 8 -> 4 -> 2 -> 1. All bf16 2x-mode wide adds.
            s = c % 2
            po = pt_sb[:, s, :, :].rearrange("p t m -> p (t m)")
            do = dscr[:].rearrange("p t m -> p (t m)")
            E = 8 * MCH
            nc.vector.tensor_add(do[:, 0:E], po[:, 0:E], po[:, E:2 * E])
            nc.vector.tensor_add(do[:, 0:E], do[:, 0:E], po[:, 2 * E:3 * E])
            nc.vector.tensor_add(do[:, 0:E], do[:, 0:E], po[:, 3 * E:4 * E])
            nc.vector.tensor_add(do[:, 0:E // 2], do[:, 0:E // 2],
                                 do[:, E // 2:E])
            nc.vector.tensor_add(do[:, 0:E // 4], do[:, 0:E // 4],
                                 do[:, E // 4:E // 2])
            nc.vector.tensor_add(do[:, 0:MCH], do[:, 0:MCH],
                                 do[:, MCH:2 * MCH])

        def den_mm(c):
            # ones^T @ acc: reduces over partitions AND broadcasts row-sums
            den_ps = psS.tile([P, 3, MCH], F32, tag="ps", name="ps_den")
            nc.tensor.matmul(den_ps[:, 0, :], ones_sb[:], dscr[:, 0, :],
                             start=True, stop=True)
            return den_ps

        def recip_of(den_ps):
            r = rcp.tile([P, MCH], F32, tag="rc")
            nc.vector.reciprocal_approx_fast(out=r[:], in_=den_ps[:, 0, :])
            return r

        def ep_mul(pout, r, co):
            o = osb.tile([P, MCH], F32, tag="osb")
            nc.vector.tensor_mul(o[:], pout[:, co, :], r[:])
            return o

        def ep_add_dma(c, o, co, q=nc.sync):
            nc.vector.tensor_add(o[:], o[:], ab2_sb[:, co, ts(c, MCH)])
            q.dma_start(out_v[:, co, ts(c, MCH)], o[:])

        def ab2():
            for co in range(2):
                nc.vector.tensor_scalar_add(ab2_sb[:, co, :], a_sb[:, co, :],
                                            bias_sb[:, 2 + co:3 + co])

        # ---------- prefix: projections + S/exp + VP tiles 0..23 of
        # chunk 0. s-groups trail the k eighths that complete their
        # tiles; vp rides two eighths behind.
        sgroups_at = {0: [], 1: [0], 2: [1, 2], 3: [3, 4], 4: [5],
                      5: [6, 7], 6: [8], 7: [9, 10]}
        out_ps0 = psOut.tile([P, 2, MCH], F32, tag="out")
        for e in range(8):
            psA = psS.tile([P, 3, MCH], F32, tag="ps", name="ps_proj")
            proj_k(e, psA[:, 0, :])
            proj_vt_quad(e, psA[:, 1:3, :])
            if e == 1:
                psQ = psS.tile([P, 3, MCH], F32, tag="ps", name="ps_q")
                proj_q(0, psQ[:, 0, :])
            elif e == 4:
                psQ = psS.tile([P, 3, MCH], F32, tag="ps", name="ps_q")
                proj_q(1, psQ[:, 0, :])
            for g in sgroups_at[e]:
                s_group(0, g)
            if e >= 2:
                for t in range(4 * (e - 2), 4 * (e - 2) + 4):
                    vp(0, t, out_ps0)

        # ---------- phases X_c: VP(c) + S/exp(c+1) + den(c) + ep(c) ----
        st = {"out": out_ps0}
        for c in range(NCH):
            # deferred S groups of chunk c: PE work that covers the
            # ep_mul(c-1) drain of the psOut slot.
            if c >= 1:
                s_group(c, 8)
                s_group(c, 9)
                s_group(c, 10)
            if 1 <= c < NCH - 1:
                psQ = psS.tile([P, 3, MCH], F32, tag="ps", name="ps_q")
                proj_q(c + 1, psQ[:, 0, :])
            if c == 0:
                ab2()
            den_tree(c)
            if c == 0:
                out_ps = st.pop("out")
            else:
                out_ps = psOut.tile([P, 2, MCH], F32, tag="out")
            jstart = 12 if c == 0 else 0

            if c < NCH - 1:
                nxt = 0
                for j in range(jstart, NT // 2):
                    vp(c, 2 * j, out_ps)
                    vp(c, 2 * j + 1, out_ps, last=(j == NT // 2 - 1))
                    # spread chunk c+1's groups 0..7 over the vp iters
                    want = (j - jstart + 1) * 8 // (NT // 2 - jstart)
                    while nxt < min(want, 8):
                        s_group(c + 1, nxt)
                        nxt += 1
                    if j == (13 if c == 0 else 10):
                        st["den"] = den_mm(c)
                        st["rc"] = recip_of(st.pop("den"))
                rc = st.pop("rc")
                o0 = ep_mul(out_ps, rc, 0)
                o1 = ep_mul(out_ps, rc, 1)
                ep_add_dma(c, o0, 0, q=nc.sync)
                ep_add_dma(c, o1, 1, q=nc.scalar)
            else:
                # final chunk: co-major so the co0 epilogue + DMA hide
                # under co1's matmuls.
                for t in range(NT):
                    vp(c, t, out_ps, co=0, last=(t == NT - 1))
                    if t == 20:
                        st["den"] = den_mm(c)
                        st["rc"] = recip_of(st.pop("den"))
                rc = st["rc"]
                o0 = ep_mul(out_ps, rc, 0)
                ep_add_dma(c, o0, 0, q=nc.sync)
                for t in range(NT):
                    vp(c, t, out_ps, co=1, last=(t == NT - 1))
                rc = st.pop("rc")
                o1 = ep_mul(out_ps, rc, 1)
                ep_add_dma(c, o1, 1, q=nc.scalar)


def _build():
    nc = bacc.Bacc("TRN2", target_bir_lowering=False, debug=False)
    a_d = nc.dram_tensor("a_s", [P, 2, MS], BF16, kind="ExternalInput").ap()
    p_d = nc.dram_tensor("p_s", [P, 2, HW], BF16, kind="ExternalInput").ap()
    wqk_d = nc.dram_tensor("wqk", [P, 2, 2 * CH], BF16, kind="ExternalInput").ap()
    wvt_d = nc.dram_tensor("wvt", [P, 2, C], BF16, kind="ExternalInput").ap()
    bias_d = nc.dram_tensor("biasb", [P, 4], F32, kind="ExternalInput").ap()
    out_d = nc.dram_tensor("out_s", [C, MS], F32, kind="ExternalOutput").ap()
    with tile.TileContext(nc) as tc:
        _emit(tc, out_d, a_d, p_d, wqk_d, wvt_d, bias_d)
    nc.compile()
    return nc


def _get_prog():
    global _PROG
    if _PROG is None:
        _PROG = _build()
    return _PROG


def _ci_co(x):
    # [C, M] -> [ci, co, M] with C = co*128 + ci, cast bf16
    m = x.shape[1]
    return np.ascontiguousarray(
        x.reshape(2, P, m).transpose(1, 0, 2).astype(BT))


def kernel(**inputs):
    a = np.ascontiguousarray(np.asarray(inputs["a"], dtype=np.float32)).reshape(
        B, C, HW
    )
    p = np.ascontiguousarray(np.asarray(inputs["p"], dtype=np.float32)).reshape(
        B, C, HW
    )
    wqt = _ci_co(np.asarray(inputs["Wq"], dtype=np.float32).T)
    wkt = _ci_co(np.asarray(inputs["Wk"], dtype=np.float32).T)
    wqk = np.ascontiguousarray(np.concatenate([wqt, wkt], axis=2))
    wvt = _ci_co(np.asarray(inputs["Wv"], dtype=np.float32).T)
    bq = np.asarray(inputs["bq"], dtype=np.float32).reshape(CH, 1)
    bk = np.asarray(inputs["bk"], dtype=np.float32).reshape(CH, 1)
    bv = np.asarray(inputs["bv"], dtype=np.float32).reshape(2, P).T
    bias = np.ascontiguousarray(np.concatenate([bq, bk, bv], axis=1))

    nc = _get_prog()
    in_maps = []
    for core in range(NCORES):
        b, h = divmod(core, 2)
        in_maps.append(
            {
                "a_s": _ci_co(a[b, :, h * MS : (h + 1) * MS]),
                "p_s": _ci_co(p[b]),
                "wqk": wqk,
                "wvt": wvt,
                "biasb": bias,
            }
        )
    kwargs = {}
    if TRACE:
        kwargs["trace"] = True
        if TMPDIR:
            kwargs["tmpdir"] = TMPDIR
    res = run_bass_kernel_spmd(nc, in_maps, core_ids=list(range(NCORES)), **kwargs)
    global LAST_RESULT
    LAST_RESULT = res

    out = np.empty((B, C, HW), dtype=np.float32)
    for core in range(NCORES):
        b, h = divmod(core, 2)
        out[b, :, h * MS : (h + 1) * MS] = res.results[core]["out_s"]
    return out.reshape(B, C, H, W)


# revision 13
# speedup vs baseline: 1.0310x; 1.0310x over previous
"""Trainium2 Bass kernel for nn_CrossAttention_9174050144362.

Reference computation (per batch b, spatial flattened to hw=4096):
    Q = Wq @ a + bq      [128, 4096]
    K = Wk @ p + bk      [128, 4096]
    V = Wv @ p + bv      [256, 4096]
    attn = softmax_n(Q^T K)            [4096, 4096]
    out  = V @ attn^T + a              [256, 4096]

Sharding: 8 cores = (4 batches) x (2 query halves of 2048). Each core
computes full K/V for its batch and attends its 2048 queries against all
4096 keys. No collectives.

v6 schedule (from v5):
  * All matmul operands bf16 (host-cast): 3.2MB input stream, 113ns
    LDWEIGHTS, rel err ~3e-3 (vs 2e-2 budget).
  * Chunk 0's VP is folded into the prefix: vp(0, 4(e-2)..4(e-2)+3)
    rides each eighth-iteration, so the input-DMA-bound prefix idle
    fills with phase-0 work; phase 0 keeps only tiles 24..31.
  * Rings: sync [wqk, p q0, q2], gpsimd [p q1, q3] (pure p - quarter 1
    no longer lands behind wvt), scalar [bias, wvt, a x3].
  * S tiles exp'd in TRIPLES ([P,1536], 11 groups/chunk): ACT ~65us.
  * Final phase runs co-major (all co0 VP matmuls, then co1) so the
    co0 epilogue + out DMA hide under co1's matmuls.
  * Denominator: 6 wide out-of-place bf16 adds into a scratch, den
    matmul + 1/x mid-phase; boundary = ep_mul only, covered by the
    next chunk's deferred S groups. PSUM: 2 S slots x 3 + out x 2 = 8.
"""

import numpy as np
import ml_dtypes

import concourse.bass as bass
import concourse.tile as tile
from concourse import bacc, mybir
from concourse.bass_utils import run_bass_kernel_spmd

B, C, H, W = 4, 256, 64, 64
HW = H * W            # 4096 keys
CH = C // 2           # 128 q/k channels
P = 128               # partitions
MS = HW // 2          # 2048 queries per core
MCH = 512             # query chunk (PSUM-bank sized)
NT = HW // P          # 32 key tiles
NCH = MS // MCH       # 4 query chunks
NG = (NT + 2) // 3    # 11 S-exp groups per chunk (3 tiles each, last 2)
NCORES = 8

F32 = mybir.dt.float32
BF16 = mybir.dt.bfloat16
AF = mybir.ActivationFunctionType
BT = ml_dtypes.bfloat16

TRACE = False
TMPDIR = None
LAST_RESULT = None

_PROG = None


def _emit(tc, out_d, a_d, p_d, wqk_d, wvt_d, bias_d):
    nc = tc.nc
    ts = bass.ts

    with (
        tc.tile_pool(name="statics", bufs=1) as statics,
        tc.tile_pool(name="rcp", bufs=2) as rcp,
        tc.tile_pool(name="osb", bufs=4) as osb,
        tc.tile_pool(name="psS", bufs=2, space="PSUM") as psS,
        tc.tile_pool(name="psOut", bufs=1, space="PSUM") as psOut,
    ):
        # ---- statics: wqk at the sync head, wvt + biases on scalar
        wqk_sb = statics.tile([P, 2, 2 * CH], BF16)
        nc.sync.dma_start(wqk_sb[:], wqk_d[:])
        bias_sb = statics.tile([P, 4], F32)
        nc.scalar.dma_start(bias_sb[:], bias_d[:])
        wvt_sb = statics.tile([P, 2, C], BF16)
        nc.scalar.dma_start(wvt_sb[:], wvt_d[:])
        ones_sb = statics.tile([P, P], BF16)
        nc.gpsimd.memset(ones_sb[:], 1.0)

        # ---- bulk inputs (bf16). p quarters alternate sync/gpsimd
        # rings; a in 3 pieces (512, 512, 1024 queries) on scalar.
        a_sb = statics.tile([P, 4, 2, MCH], BF16)    # piece-major
        p_sb = statics.tile([P, 4, 2, HW // 4], BF16)  # quarter-major
        for i in (0, 2):
            nc.gpsimd.dma_start(p_sb[:, i, :, :], p_d[i])
        for i in (1, 3):
            nc.sync.dma_start(p_sb[:, i, :, :], p_d[i])
        for i in range(4):
            nc.scalar.dma_start(a_sb[:, i, :, :], a_d[i])

        # ---- persistent SBUF state
        q_sb = statics.tile([P, MS], BF16)
        k_sb = statics.tile([P, HW], BF16)
        vt_sb = statics.tile([P, NT, C], BF16)
        pt_sb = statics.tile([P, 2, NT, MCH], BF16)
        dscr = statics.tile([P, 8, MCH], BF16)     # den tree scratch
        ab2_sb = statics.tile([P, 2, MS], BF16)    # a + bv (residual)

        out_v = out_d.rearrange("(co ci) m -> ci co m", ci=P)

        # ---------- helpers ----------
        def proj_q(c, half):
            for co in range(2):
                nc.tensor.matmul(half[:], wqk_sb[:, co, 0:CH],
                                 a_sb[:, c, co, :],
                                 start=(co == 0), stop=(co == 1))
            nc.vector.tensor_scalar_add(q_sb[:, ts(c, MCH)], half[:],
                                        bias_sb[:, 0:1])

        def proj_k(e, half):
            E8 = HW // 8
            for co in range(2):
                nc.tensor.matmul(half[:], wqk_sb[:, co, CH:2 * CH],
                                 p_sb[:, e // 2, co, ts(e % 2, E8)],
                                 start=(co == 0), stop=(co == 1))
            nc.vector.tensor_scalar_add(k_sb[:, ts(e, E8)], half[:],
                                        bias_sb[:, 1:2])

        def proj_vt_quad(e, vq):
            # V^T tiles 4e..4e+3 into banks 1-2 of a psS slot; ONE DVE
            # eviction. vt[n, c] = sum_ci p[ci, n] * WvT[ci, c]  (no bias;
            # bv folded into ab2 since attn rows sum to 1)
            vq_f = vq.rearrange("p a b -> p (a b)")
            for i in range(4):
                t = 4 * e + i
                for co in range(2):
                    nc.tensor.matmul(vq_f[:, ts(i, C)],
                                     p_sb[:, t // 8, co, ts(t % 8, P)],
                                     wvt_sb[:, co, :],
                                     start=(co == 0), stop=(co == 1))
            nc.vector.tensor_copy(
                vt_sb[:, 4 * e : 4 * e + 4, :],
                vq_f.rearrange("p (t c) -> p t c", t=4))

        def s_group(c, g):
            # S tiles 3g..3g+2 -> one [P,3,512] PSUM slot, ONE exp over
            # [P,1536] into the pt store.
            s = c % 2
            t0, t1 = 3 * g, min(3 * g + 3, NT)
            ps = psS.tile([P, 3, MCH], F32, tag="ps", name="ps_s")
            for t in range(t0, t1):
                nc.tensor.matmul(ps[:, t - t0, :], k_sb[:, ts(t, P)],
                                 q_sb[:, ts(c, MCH)], start=True, stop=True)
            nc.scalar.activation(pt_sb[:, s, t0:t1, :], ps[:, 0:t1 - t0, :],
                                 AF.Exp)

        def vp(c, t, out_ps, co=None, last=False):
            s = c % 2
            for co_ in ([0, 1] if co is None else [co]):
                nc.tensor.matmul(out_ps[:, co_, :],
                                 vt_sb[:, t, ts(co_, P)], pt_sb[:, s, t, :],
                                 start=(t == 0), stop=last)

        def den_tree(c):
            # dscr[0:8] = sum of the 4 tile-octs (pure reads of pt),
            # then fold 8 -> 4 -> 2 -> 1. All bf16 2x-mode wide adds.
            s = c % 2
            po = pt_sb[:, s, :, :].rearrange("p t m -> p (t m)")
            do = dscr[:].rearrange("p t m -> p (t m)")
            E = 8 * MCH
            nc.vector.tensor_add(do[:, 0:E], po[:, 0:E], po[:, E:2 * E])
            nc.vector.tensor_add(do[:, 0:E], do[:, 0:E], po[:, 2 * E:3 * E])
            nc.vector.tensor_add(do[:, 0:E], do[:, 0:E], po[:, 3 * E:4 * E])
            nc.vector.tensor_add(do[:, 0:E // 2], do[:, 0:E // 2],
                                 do[:, E // 2:E])
            nc.vector.tensor_add(do[:, 0:E // 4], do[:, 0:E // 4],
                                 do[:, E // 4:E // 2])
            nc.vector.tensor_add(do[:, 0:MCH], do[:, 0:MCH],
                                 do[:, MCH:2 * MCH])

        def den_mm(c):
            # ones^T @ acc: reduces over partitions AND broadcasts row-sums
            den_ps = psS.tile([P, 3, MCH], F32, tag="ps", name="ps_den")
            nc.tensor.matmul(den_ps[:, 0, :], ones_sb[:], dscr[:, 0, :],
                             start=True, stop=True)
            return den_ps

        def recip_of(den_ps):
            r = rcp.tile([P, MCH], F32, tag="rc")
            nc.vector.reciprocal_approx_fast(out=r[:], in_=den_ps[:, 0, :])
            return r

        def ep_mul(pout, r, co):
            o = osb.tile([P, MCH], F32, tag="osb")
            nc.vector.tensor_mul(o[:], pout[:, co, :], r[:])
            return o

        def ep_add_dma(c, o, co, q=nc.sync):
            nc.vector.tensor_add(o[:], o[:], ab2_sb[:, co, ts(c, MCH)])
            q.dma_start(out_v[:, co, ts(c, MCH)], o[:])

        def ab2():
            for co in range(2):
                nc.vector.tensor_scalar_add(
                    ab2_sb[:, co, :].rearrange("p (pc m) -> p pc m", pc=4),
                    a_sb[:, :, co, :],
                    bias_sb[:, 2 + co:3 + co])

        # ---------- prefix: projections + S/exp + VP tiles 0..23 of
        # chunk 0. s-groups trail the k eighths that complete their
        # tiles; vp rides two eighths behind.
        sgroups_at = {0: [], 1: [0], 2: [1, 2], 3: [3, 4], 4: [5],
                      5: [6, 7], 6: [8], 7: [9, 10]}
        out_ps0 = psOut.tile([P, 2, MCH], F32, tag="out")
        for e in range(8):
            psA = psS.tile([P, 3, MCH], F32, tag="ps", name="ps_proj")
            proj_k(e, psA[:, 0, :])
            proj_vt_quad(e, psA[:, 1:3, :])
            if e == 1:
                psQ = psS.tile([P, 3, MCH], F32, tag="ps", name="ps_q")
                proj_q(0, psQ[:, 0, :])
            elif e == 4:
                psQ = psS.tile([P, 3, MCH], F32, tag="ps", name="ps_q")
                proj_q(1, psQ[:, 0, :])
            for g in sgroups_at[e]:
                s_group(0, g)
            if e >= 2:
                for t in range(4 * (e - 2), 4 * (e - 2) + 4):
                    vp(0, t, out_ps0)

        # ---------- phases X_c: VP(c) + S/exp(c+1) + den(c) + ep(c) ----
        st = {"out": out_ps0}
        for c in range(NCH):
            # deferred S groups of chunk c: PE work that covers the
            # ep_mul(c-1) drain of the psOut slot.
            if c >= 1:
                s_group(c, 8)
                s_group(c, 9)
                s_group(c, 10)
            if 1 <= c < NCH - 1:
                psQ = psS.tile([P, 3, MCH], F32, tag="ps", name="ps_q")
                proj_q(c + 1, psQ[:, 0, :])
            if c == 0:
                ab2()
            den_tree(c)
            if c == 0:
                out_ps = st.pop("out")
            else:
                out_ps = psOut.tile([P, 2, MCH], F32, tag="out")
            jstart = 12 if c == 0 else 0

            if c < NCH - 1:
                nxt = 0
                for j in range(jstart, NT // 2):
                    vp(c, 2 * j, out_ps)
                    vp(c, 2 * j + 1, out_ps, last=(j == NT // 2 - 1))
                    # spread chunk c+1's groups 0..7 over the vp iters
                    want = (j - jstart + 1) * 8 // (NT // 2 - jstart)
                    while nxt < min(want, 8):
                        s_group(c + 1, nxt)
                        nxt += 1
                    if j == (13 if c == 0 else 10):
                        st["den"] = den_mm(c)
                        st["rc"] = recip_of(st.pop("den"))
                rc = st.pop("rc")
                o0 = ep_mul(out_ps, rc, 0)
                o1 = ep_mul(out_ps, rc, 1)
                ep_add_dma(c, o0, 0, q=nc.sync)
                ep_add_dma(c, o1, 1, q=nc.scalar)
            else:
                # final chunk: co-major so the co0 epilogue + DMA hide
                # under co1's matmuls.
                for t in range(NT):
                    vp(c, t, out_ps, co=0, last=(t == NT - 1))
                    if t == 20:
                        st["den"] = den_mm(c)
                        st["rc"] = recip_of(st.pop("den"))
                rc = st["rc"]
                o0 = ep_mul(out_ps, rc, 0)
                ep_add_dma(c, o0, 0, q=nc.sync)
                for t in range(NT):
                    vp(c, t, out_ps, co=1, last=(t == NT - 1))
                rc = st.pop("rc")
                o1 = ep_mul(out_ps, rc, 1)
                ep_add_dma(c, o1, 1, q=nc.scalar)


def _build():
    nc = bacc.Bacc("TRN2", target_bir_lowering=False, debug=False)
    a_d = nc.dram_tensor("a_s", [4, P, 2, MCH], BF16, kind="ExternalInput").ap()
    p_d = nc.dram_tensor("p_s", [4, P, 2, HW // 4], BF16, kind="ExternalInput").ap()
    wqk_d = nc.dram_tensor("wqk", [P, 2, 2 * CH], BF16, kind="ExternalInput").ap()
    wvt_d = nc.dram_tensor("wvt", [P, 2, C], BF16, kind="ExternalInput").ap()
    bias_d = nc.dram_tensor("biasb", [P, 4], F32, kind="ExternalInput").ap()
    out_d = nc.dram_tensor("out_s", [C, MS], F32, kind="ExternalOutput").ap()
    with tile.TileContext(nc) as tc:
        _emit(tc, out_d, a_d, p_d, wqk_d, wvt_d, bias_d)
    nc.compile()
    return nc


def _get_prog():
    global _PROG
    if _PROG is None:
        _PROG = _build()
    return _PROG


def _piece(x3, plen):
    # [P, 2, M] -> [M//plen, P, 2, plen] piece-major contiguous
    n = x3.shape[2] // plen
    return np.ascontiguousarray(
        x3.reshape(P, 2, n, plen).transpose(2, 0, 1, 3))


def _ci_co(x):
    # [C, M] -> [ci, co, M] with C = co*128 + ci, cast bf16
    m = x.shape[1]
    return np.ascontiguousarray(
        x.reshape(2, P, m).transpose(1, 0, 2).astype(BT))


def kernel(**inputs):
    a = np.ascontiguousarray(np.asarray(inputs["a"], dtype=np.float32)).reshape(
        B, C, HW
    )
    p = np.ascontiguousarray(np.asarray(inputs["p"], dtype=np.float32)).reshape(
        B, C, HW
    )
    wqt = _ci_co(np.asarray(inputs["Wq"], dtype=np.float32).T)
    wkt = _ci_co(np.asarray(inputs["Wk"], dtype=np.float32).T)
    wqk = np.ascontiguousarray(np.concatenate([wqt, wkt], axis=2))
    wvt = _ci_co(np.asarray(inputs["Wv"], dtype=np.float32).T)
    bq = np.asarray(inputs["bq"], dtype=np.float32).reshape(CH, 1)
    bk = np.asarray(inputs["bk"], dtype=np.float32).reshape(CH, 1)
    bv = np.asarray(inputs["bv"], dtype=np.float32).reshape(2, P).T
    bias = np.ascontiguousarray(np.concatenate([bq, bk, bv], axis=1))

    nc = _get_prog()
    in_maps = []
    for core in range(NCORES):
        b, h = divmod(core, 2)
        in_maps.append(
            {
                "a_s": _piece(_ci_co(a[b, :, h * MS : (h + 1) * MS]), MCH),
                "p_s": _piece(_ci_co(p[b]), HW // 4),
                "wqk": wqk,
                "wvt": wvt,
                "biasb": bias,
            }
        )
    kwargs = {}
    if TRACE:
        kwargs["trace"] = True
        if TMPDIR:
            kwargs["tmpdir"] = TMPDIR
    res = run_bass_kernel_spmd(nc, in_maps, core_ids=list(range(NCORES)), **kwargs)
    global LAST_RESULT
    LAST_RESULT = res

    out = np.empty((B, C, HW), dtype=np.float32)
    for core in range(NCORES):
        b, h = divmod(core, 2)
        out[b, :, h * MS : (h + 1) * MS] = res.results[core]["out_s"]
    return out.reshape(B, C, H, W)
